# revision 6
# baseline (speedup 1.0000x reference)
"""AttentionPooling Trainium2 kernel, v3: fully streaming single-table pipeline.

vs v2: no act-table phase blocks (exp computed from tanh, which shares the
gelu table set), software-pipelined scatter (e-group g-1's A-build + scatter
matmuls interleave with e-group g's MLP work), A-build batched 16 chunks per
DVE op via stride-0 broadcast APs (is_equal on DVE, e-multiply on GpSimd),
mm2 logits accumulate per-supertile in PSUM and feed ACT tanh directly.

  e-group = 4 supertiles = 64 chunks.
  e = exp(l + b2) = (1 + t)/(1 - t),  t = tanh((l + b2)/2).
"""

import os

import numpy as np
import ml_dtypes

N_TOTAL = 1_000_000
D = 256
G = 8192
NCORES = 8
SEG_PER_CORE = G // NCORES   # 1024
P = 128
ROWL = D + 1                 # 257
GRP_SEGS = 64                # segments per accumulator group
N_GRP = SEG_PER_CORE // GRP_SEGS  # 16
CH_PER_TILE = 16             # chunks per supertile (DMA granularity)
ST_PER_EG = 4                # supertiles per e-group
CH_PER_EG = CH_PER_TILE * ST_PER_EG  # 64
MLP_BATCH = 4                # chunks per mm1 batch (512 rows)

XR_POOL_FP16 = bool(int(os.environ.get("XR_POOL_FP16", "0")))
AMULT_ON_GPSIMD = bool(int(os.environ.get("AMULT_ON_GPSIMD", "1")))

LAST_EXEC_NS = None


def _plan_from_batch(batch):
    batch = np.asarray(batch).astype(np.int64)
    gb = np.searchsorted(batch, np.arange(0, G + 1, GRP_SEGS))
    rows = np.diff(gb).reshape(NCORES, N_GRP)
    nch = np.ceil(rows / P).astype(np.int64).max(axis=0)
    assert rows.min() > 0
    n_chunks = int(nch.sum())
    pad = (-n_chunks) % CH_PER_EG
    nch[-1] += pad
    n_chunks += pad
    grp_of_chunk = np.repeat(np.arange(N_GRP), nch)
    first_c = np.searchsorted(grp_of_chunk, np.arange(N_GRP), "left")
    last_c = np.searchsorted(grp_of_chunk, np.arange(N_GRP), "right") - 1
    return gb, nch, n_chunks, grp_of_chunk, first_c, last_c


def _prep_inputs(x, batch):
    xr_dt = np.float16 if XR_POOL_FP16 else ml_dtypes.float8_e3m4
    xt_dt = ml_dtypes.float8_e4m3
    batch = np.asarray(batch).astype(np.int64)
    x = np.asarray(x, dtype=np.float32)
    gb, nch, n_chunks, grp_of_chunk, first_c, last_c = _plan_from_batch(batch)
    n_rows = n_chunks * P
    n_batches = n_chunks // MLP_BATCH

    xr_list, xt_list, lbt_list = [], [], []
    for k in range(NCORES):
        xpad = np.zeros((n_rows, D), np.float32)
        lbp = np.full(n_chunks * P, -1000.0, np.float32)
        ofs = 0
        for j in range(N_GRP):
            b0 = int(gb[k * N_GRP + j])
            b1 = int(gb[k * N_GRP + j + 1])
            nr = b1 - b0
            xpad[ofs : ofs + nr] = x[b0:b1]
            lbp[ofs : ofs + nr] = batch[b0:b1] - (SEG_PER_CORE * k + GRP_SEGS * j)
            ofs += int(nch[j]) * P
        assert ofs == n_rows

        xr = np.empty((P, n_chunks, ROWL), xr_dt)
        xr[:, :, :D] = xpad.reshape(n_chunks, P, D).transpose(1, 0, 2).astype(xr_dt)
        xr[:, :, D] = 1.0
        xr_list.append(np.ascontiguousarray(xr.reshape(P, n_chunks * ROWL)))

        x8 = xpad.astype(xt_dt)
        xt = np.ascontiguousarray(
            x8.reshape(n_batches, MLP_BATCH * P, 2, P).transpose(3, 0, 2, 1)
        )
        xt_list.append(xt.reshape(P, n_batches * 2 * MLP_BATCH * P))

        lbt_list.append(
            np.ascontiguousarray(lbp.reshape(n_chunks, P).T).astype(np.float16)
        )
    return xr_list, xt_list, lbt_list, n_chunks, grp_of_chunk, first_c, last_c


def _build_kernel(n_chunks, grp_of_chunk, first_c, last_c):
    from contextlib import ExitStack

    import concourse.bass as bass
    import concourse.tile as tile
    from concourse import bacc, mybir

    f32 = mybir.dt.float32
    f16 = mybir.dt.float16
    f8e4 = mybir.dt.float8e4
    f8e3 = mybir.dt.float8e3
    xr_t = f16 if XR_POOL_FP16 else f8e3
    AF = mybir.ActivationFunctionType
    OP = mybir.AluOpType
    PM = mybir.MatmulPerfMode

    nc = bacc.Bacc(
        "TRN2",
        target_bir_lowering=False,
        debug=False,
        enable_asserts=False,
        num_devices=NCORES,
    )

    n_tiles = n_chunks // CH_PER_TILE
    n_batches = n_chunks // MLP_BATCH
    n_eg = n_chunks // CH_PER_EG
    XR_ST = CH_PER_TILE * ROWL
    XT_ST = (CH_PER_TILE // MLP_BATCH) * 2 * MLP_BATCH * P

    xr_d = nc.dram_tensor("xr", [P, n_chunks * ROWL], xr_t, kind="ExternalInput").ap()
    xt_d = nc.dram_tensor("xt", [P, n_batches * 2 * MLP_BATCH * P], f8e4,
                          kind="ExternalInput").ap()
    lbt_d = nc.dram_tensor("lbt", [P, n_chunks], f16, kind="ExternalInput").ap()
    w1dr_d = nc.dram_tensor("w1dr", [P, 2 * P], f8e4, kind="ExternalInput").ap()
    w2_d = nc.dram_tensor("w2", [P, 1], f16, kind="ExternalInput").ap()
    b1_d = nc.dram_tensor("b1v", [P, 1], f32, kind="ExternalInput").ap()
    b2h_d = nc.dram_tensor("b2h", [P, 1], f32, kind="ExternalInput").ap()
    iota_d = nc.dram_tensor("iota16", [P, CH_PER_TILE * GRP_SEGS], f16,
                            kind="ExternalInput").ap()
    out_d = nc.dram_tensor("pooled", [SEG_PER_CORE, D], f32,
                           kind="ExternalOutput").ap()

    with tile.TileContext(nc) as tc, ExitStack() as ctx:
        cpool = ctx.enter_context(tc.tile_pool(name="const", bufs=1))
        xrpool = ctx.enter_context(tc.tile_pool(name="xrpool", bufs=4 * ST_PER_EG + 2))
        xtpool = ctx.enter_context(tc.tile_pool(name="xtpool", bufs=10))
        hpool = ctx.enter_context(tc.tile_pool(name="hpool", bufs=3))
        tpool = ctx.enter_context(tc.tile_pool(name="tpool", bufs=2))
        mpool = ctx.enter_context(tc.tile_pool(name="mpool", bufs=6))
        apool = ctx.enter_context(tc.tile_pool(name="apool", bufs=3 * ST_PER_EG + 2))
        opool = ctx.enter_context(tc.tile_pool(name="opool", bufs=2))
        psH = ctx.enter_context(tc.tile_pool(name="psH", bufs=2, space="PSUM"))
        psL = ctx.enter_context(tc.tile_pool(name="psL", bufs=2, space="PSUM"))
        psA = ctx.enter_context(tc.tile_pool(name="psA", bufs=2, space="PSUM"))

        w1dr_sb = cpool.tile([P, 2 * P], f8e4, tag="w1dr", name="w1dr_sb")
        nc.sync.dma_start(w1dr_sb[:], w1dr_d)
        w1dr = w1dr_sb.rearrange("p (s m) -> p s m", s=2)
        w2 = cpool.tile([P, 1], f16, tag="w2", name="w2")
        nc.sync.dma_start(w2[:], w2_d)
        b1v = cpool.tile([P, 1], f32, tag="b1v", name="b1v")
        nc.sync.dma_start(b1v[:], b1_d)
        b2h = cpool.tile([P, 1], f32, tag="b2h", name="b2h")
        nc.sync.dma_start(b2h[:], b2h_d)
        iota16 = cpool.tile([P, CH_PER_TILE * GRP_SEGS], f16, tag="iota",
                            name="iota16")
        nc.sync.dma_start(iota16[:], iota_d)
        lbt = cpool.tile([P, n_chunks], f16, tag="lbt", name="lbt")
        nc.sync.dma_start(lbt[:], lbt_d)

        acc = {}
        xt_cur = {}
        pend = []       # scatter closures from e-group eg-2 (being consumed)
        wait1 = []      # scatter closures from e-group eg-1 (aging)
        pi = 0          # next pending closure to emit

        def emit_pending(k):
            nonlocal pi
            for _ in range(k):
                if pi < len(pend):
                    pend[pi]()
                    pi += 1

        def dma_xt(t):
            xt_tile = xtpool.tile([P, XT_ST], f8e4, tag="xt")
            nc.gpsimd.dma_start(xt_tile[:], xt_d[:, t * XT_ST : (t + 1) * XT_ST])
            return xt_tile

        for eg in range(n_eg):
            xrt = {}
            new_pend = []
            if eg == 0:
                for st in range(ST_PER_EG):
                    xt_cur[st] = dma_xt(st)
            xt_next = {}
            if eg + 1 < n_eg:
                for st in range(ST_PER_EG):
                    xt_next[st] = dma_xt((eg + 1) * ST_PER_EG + st)
            tg = tpool.tile([P, CH_PER_EG], f32, tag="tg", bufs=2, name="tg")
            M16s = {}
            for st in range(ST_PER_EG):
                t = eg * ST_PER_EG + st
                xr_tile = xrpool.tile([P, XR_ST], xr_t, tag="xr")
                nc.sync.dma_start(xr_tile[:], xr_d[:, t * XR_ST : (t + 1) * XR_ST])
                xrt[st] = xr_tile
                # one-hot mask for this supertile (depends only on lbt)
                c0m = t * CH_PER_TILE
                lb_b = lbt[:, c0m : c0m + CH_PER_TILE].broadcast_to(
                    [P, CH_PER_TILE, GRP_SEGS]
                )
                M16 = mpool.tile([P, CH_PER_TILE * GRP_SEGS], f16, tag="M16")
                nc.vector.tensor_tensor(
                    M16.rearrange("p (c g) -> p c g", c=CH_PER_TILE),
                    iota16.rearrange("p (c g) -> p c g", c=CH_PER_TILE),
                    lb_b, OP.is_equal,
                )
                M16s[st] = M16
                xtv = xt_cur[st].rearrange("p (b s n) -> p b s n",
                                           b=CH_PER_TILE // MLP_BATCH, s=2)

                lg_ps = psL.tile([P, CH_PER_TILE], f32, tag="lg", name="lg_ps")
                W = MLP_BATCH * P
                for bp in range(CH_PER_TILE // MLP_BATCH // 2):
                    hT2_ps = psH.tile([P, 2 * W], f32, tag="h", name="h_ps")
                    for half in range(2):
                        b = 2 * bp + half
                        nc.tensor.matmul(hT2_ps[:, half * W : (half + 1) * W],
                                         w1dr, xtv[:, b, :, :],
                                         start=True, stop=True,
                                         perf_mode=PM.DoubleRow)
                    hT2 = hpool.tile([P, 2 * W], f16, tag="hT")
                    nc.scalar.activation(hT2[:], hT2_ps[:], AF.Gelu, bias=b1v[:])
                    for ci in range(2 * MLP_BATCH):
                        nc.tensor.matmul(
                            lg_ps[:, bp * 2 * MLP_BATCH + ci
                                  : bp * 2 * MLP_BATCH + ci + 1],
                            hT2[:, ci * P : (ci + 1) * P], w2[:],
                            start=True, stop=True,
                        )
                    emit_pending(10)
                # t = tanh((l + b2)/2), straight from PSUM into tg slice
                nc.scalar.activation(
                    tg[:, st * CH_PER_TILE : (st + 1) * CH_PER_TILE],
                    lg_ps[:], AF.Tanh, bias=b2h[:], scale=0.5,
                )

            emit_pending(len(pend) - pi)  # drain any remainder

            # e for the whole e-group: ev = (1+t)/(1-t)
            num = tpool.tile([P, CH_PER_EG], f32, tag="num", bufs=2)
            nc.vector.tensor_scalar(num[:], tg[:], -1.0, 1.0, OP.mult,
                                    OP.subtract)          # -t-1 = -(1+t)
            den = tpool.tile([P, CH_PER_EG], f32, tag="den", bufs=2)
            nc.vector.tensor_scalar(den[:], tg[:], 1.0, None, OP.subtract)  # t-1
            rden = tpool.tile([P, CH_PER_EG], f32, tag="rden", bufs=2)
            nc.vector.reciprocal(rden[:], den[:])
            ev = tpool.tile([P, CH_PER_EG], f16, tag="ev", bufs=2)
            nc.vector.tensor_tensor(ev[:], num[:], rden[:], OP.mult)

            # A-build, 16 chunks per op pair
            Ats = {}
            for st in range(ST_PER_EG):
                M16 = M16s[st]
                e_b = ev[:, st * CH_PER_TILE : (st + 1) * CH_PER_TILE].broadcast_to(
                    [P, CH_PER_TILE, GRP_SEGS]
                )
                A16 = apool.tile([P, (CH_PER_TILE + 1) * GRP_SEGS], f16, tag="A16")
                eng = nc.gpsimd if AMULT_ON_GPSIMD else nc.vector
                eng.tensor_tensor(
                    A16[:, : CH_PER_TILE * GRP_SEGS].rearrange(
                        "p (c g) -> p c g", c=CH_PER_TILE),
                    M16.rearrange("p (c g) -> p c g", c=CH_PER_TILE),
                    e_b, OP.mult,
                )
                Ats[st] = A16

            # queue scatter closures for this e-group
            def make_cl(st, i, c, xr_tile, A16):
                def cl():
                    j = int(grp_of_chunk[c])
                    if j not in acc:
                        acc[j] = psA.tile([P, ROWL], f32, tag="acc",
                                          name=f"acc{j}")
                    xrv = xr_tile.rearrange("p (c n) -> p c n", c=CH_PER_TILE)
                    nc.tensor.matmul(
                        acc[j][:],
                        A16[:, i * GRP_SEGS : i * GRP_SEGS + P],
                        xrv[:, i, :],
                        start=(c == int(first_c[j])), stop=(c == int(last_c[j])),
                    )
                    if c == int(last_c[j]):
                        rz = opool.tile([GRP_SEGS, 1], f32, tag="rz", bufs=2)
                        nc.vector.reciprocal(rz[:], acc[j][:GRP_SEGS, D : D + 1])
                        ow = opool.tile([GRP_SEGS, D], f32, tag="ow", bufs=2)
                        nc.vector.tensor_scalar(ow[:], acc[j][:GRP_SEGS, :D],
                                                rz[:], None, OP.mult)
                        nc.sync.dma_start(
                            out_d[j * GRP_SEGS : (j + 1) * GRP_SEGS, :], ow[:]
                        )
                return cl

            for st in range(ST_PER_EG):
                for i in range(CH_PER_TILE):
                    c = (eg * ST_PER_EG + st) * CH_PER_TILE + i
                    new_pend.append(make_cl(st, i, c, xrt[st], Ats[st]))

            pend, pi = wait1, 0
            wait1 = new_pend
            xt_cur = xt_next

        emit_pending(len(pend) - pi)   # scatters of e-group n-2
        pend, pi = wait1, 0
        emit_pending(len(pend) - pi)   # scatters of e-group n-1

    nc.compile()
    return nc


def kernel(x, W1, b1, W2, b2, batch):
    global LAST_EXEC_NS
    from concourse import bass_utils

    xr_list, xt_list, lbt_list, n_chunks, grp_of_chunk, first_c, last_c = (
        _prep_inputs(x, batch)
    )

    W1 = np.asarray(W1, np.float32)
    b1 = np.asarray(b1, np.float32).reshape(-1)
    W2 = np.asarray(W2, np.float32).reshape(-1)
    b2 = np.asarray(b2, np.float32).reshape(-1)
    w1dr = np.ascontiguousarray(
        W1.reshape(2, P, P).transpose(1, 0, 2)
    ).astype(ml_dtypes.float8_e4m3).reshape(P, 2 * P)
    w2v = W2.reshape(P, 1).astype(np.float16)
    b1v = b1.reshape(P, 1)
    b2h = np.full((P, 1), b2[0] * 0.5, np.float32)
    iota16 = np.broadcast_to(
        np.arange(GRP_SEGS, dtype=np.float16), (P, CH_PER_TILE, GRP_SEGS)
    ).reshape(P, CH_PER_TILE * GRP_SEGS).copy()

    nc = _build_kernel(n_chunks, grp_of_chunk, first_c, last_c)

    in_maps = []
    for k in range(NCORES):
        in_maps.append(
            {
                "xr": xr_list[k],
                "xt": xt_list[k],
                "lbt": lbt_list[k],
                "w1dr": w1dr,
                "w2": w2v,
                "b1v": b1v,
                "b2h": b2h,
                "iota16": iota16,
            }
        )

    trace = bool(int(os.environ.get("KERNEL_TRACE", "0")))
    res = bass_utils.run_bass_kernel_spmd(
        nc, in_maps, core_ids=list(range(NCORES)), trace=trace
    )
    LAST_EXEC_NS = res.exec_time_ns
    out = np.concatenate([res.results[k]["pooled"] for k in range(NCORES)], axis=0)
    return out.astype(np.float32)
